# revision 1
# baseline (speedup 1.0000x reference)
"""RandomProjectionQuantizer for Trainium2, 8-core data-parallel.

Computes xq[b, n] = argmax_c <x[b,n,:] @ rp, normalize(codebook)[c,:]>
(the projection's own L2 normalization is a positive per-row scale, so it
cannot change the argmax and is skipped).

Sharding: batch dim (B=8) across the 8 cores; rp/codebook replicated.

Precision: fp16 hi/lo 3-term split matmuls (hi@hi + hi@lo + lo@hi) with
fp32 PSUM accumulation reproduces fp32 matmul results to ~1e-5 relative,
which keeps the argmax bit-stable vs the fp32 reference (0 flips in
calibration). rp and the normalized codebook are pre-scaled by 64 (exact
power of two, argmax-invariant) so their fp16 lo parts stay in normal
range.
"""

import numpy as np
from contextlib import ExitStack

B, N, D, E, C = 8, 4096, 1024, 512, 4096
P = 128
ROWS_SB = 512                 # rows per super-block (mm1 moving free dim)
N_SB = N // ROWS_SB           # 8 super-blocks per core
D_CH = D // P                 # 8 contraction chunks for mm1
E_CH = E // P                 # 4 contraction chunks for mm2
CC_W = 512                    # mm2 free-dim (one PSUM bank)
C_CH = C // CC_W              # 8 candidate chunks

_PROG = None


def _build_program():
    import concourse.bass as bass
    import concourse.tile as tile
    import concourse.masks as masks
    from concourse import bacc, mybir

    f32 = mybir.dt.float32
    f16 = mybir.dt.float16
    u32 = mybir.dt.uint32
    SUB = mybir.AluOpType.subtract
    AF = mybir.ActivationFunctionType

    nc = bacc.Bacc("TRN2", target_bir_lowering=False, debug=False)
    x_d = nc.dram_tensor("x", [N, D], f32, kind="ExternalInput")
    rp_d = nc.dram_tensor("rp", [D, E], f32, kind="ExternalInput")
    cb_d = nc.dram_tensor("cb", [C, E], f32, kind="ExternalInput")
    xq_d = nc.dram_tensor("xq", [N, 1], u32, kind="ExternalOutput")

    with tile.TileContext(nc) as tc, ExitStack() as ctx:
        const = ctx.enter_context(tc.tile_pool(name="const", bufs=1))
        persist = ctx.enter_context(tc.tile_pool(name="persist", bufs=1))

        ident = const.tile([P, P], f32)
        masks.make_identity(nc, ident[:])

        rp_hi = [persist.tile([P, E], f16, tag=f"rph{d}", name=f"rph{d}") for d in range(D_CH)]
        rp_lo = [persist.tile([P, E], f16, tag=f"rpl{d}", name=f"rpl{d}") for d in range(D_CH)]
        cnT_hi = [persist.tile([P, C], f16, tag=f"cnh{e}", name=f"cnh{e}") for e in range(E_CH)]
        cnT_lo = [persist.tile([P, C], f16, tag=f"cnl{e}", name=f"cnl{e}") for e in range(E_CH)]

        stage = ctx.enter_context(tc.tile_pool(name="stage", bufs=3))
        xin = ctx.enter_context(tc.tile_pool(name="xin", bufs=2))
        xsplit = ctx.enter_context(tc.tile_pool(name="xsplit", bufs=1))
        projp = ctx.enter_context(tc.tile_pool(name="projp", bufs=2))
        simp = ctx.enter_context(tc.tile_pool(name="simp", bufs=2))
        outp = ctx.enter_context(tc.tile_pool(name="outp", bufs=3))
        ps_tp = ctx.enter_context(
            tc.tile_pool(name="ps_tp", bufs=2, space=bass.MemorySpace.PSUM))
        ps_p1 = ctx.enter_context(
            tc.tile_pool(name="ps_p1", bufs=2, space=bass.MemorySpace.PSUM))
        ps_p2 = ctx.enter_context(
            tc.tile_pool(name="ps_p2", bufs=2, space=bass.MemorySpace.PSUM))
        ps_pro = ctx.enter_context(
            tc.tile_pool(name="ps_pro", bufs=2, space=bass.MemorySpace.PSUM))

        def rp_prologue():
            # rp -> scaled fp16 hi/lo, [d-chunk](K=d128, M=e512)
            for d in range(D_CH):
                t = stage.tile([P, E], f32, tag="cbstage", name=f"rpst{d}")
                nc.sync.dma_start(t[:], rp_d.ap()[d * P:(d + 1) * P, :])
                ts = stage.tile([P, E], f32, tag="cns", name=f"rpsc{d}")
                nc.vector.tensor_scalar_mul(ts[:], t[:], 64.0)
                nc.vector.tensor_copy(rp_hi[d][:], ts[:])
                nc.vector.tensor_tensor(rp_lo[d][:], ts[:], rp_hi[d][:], op=SUB)

        def cb_prologue(i_range):
            # codebook -> row-normalize*64 -> transpose -> fp16 hi/lo,
            # [e-chunk](K=e128, N=c4096)
            for i in i_range:
                t = stage.tile([P, E], f32, tag="cbstage", name=f"cbst{i}")
                nc.sync.dma_start(t[:], cb_d.ap()[i * P:(i + 1) * P, :])
                sq = stage.tile([P, E], f32, tag="cns", name=f"cbsq{i}")
                ssq = stage.tile([P, 1], f32, tag="cbssq", name=f"cbssq{i}")
                nc.scalar.activation(sq[:], t[:], AF.Square, accum_out=ssq[:])
                rt = stage.tile([P, 1], f32, tag="cbrt", name=f"cbrt{i}")
                # 64*rsqrt(ssq) == 1/sqrt(ssq*2^-12); exact power-of-2 scale
                nc.scalar.activation(rt[:], ssq[:], AF.Sqrt, scale=2.0 ** -12)
                scal = stage.tile([P, 1], f32, tag="cbscal", name=f"cbscal{i}")
                nc.vector.reciprocal(scal[:], rt[:])
                cns = stage.tile([P, E], f32, tag="cns", name=f"cbcns{i}")
                nc.scalar.activation(cns[:], t[:], AF.Copy, scale=scal[:, 0:1])
                pst = ps_pro.tile([P, E], f32, tag="ps_cn", name=f"pscn{i}")
                for e in range(E_CH):
                    nc.tensor.transpose(
                        pst[:, e * P:(e + 1) * P],
                        cns[:, e * P:(e + 1) * P], ident[:])
                for e in range(E_CH):
                    hi = cnT_hi[e][:, i * P:(i + 1) * P]
                    nc.vector.tensor_copy(hi, pst[:, e * P:(e + 1) * P])
                    nc.vector.tensor_tensor(
                        cnT_lo[e][:, i * P:(i + 1) * P],
                        pst[:, e * P:(e + 1) * P], hi, op=SUB)

        # ---- main loop, software-pipelined one super-block deep:
        # emit transposes+mm1 of super-block sb, then mm2+argmax of sb-1.
        # This keeps the PE stream dense: while the PE chews the long mm2
        # stretch of sb-1, the DVE split work for sb is already queued
        # ahead of sb-1's argmax ops, so the next transposes never stall.
        def load_x(sb):
            r0 = sb * ROWS_SB
            xt = []
            for j in range(ROWS_SB // P):
                t = xin.tile([P, D], f32, tag=f"x{j}", name=f"x{sb}_{j}")
                nc.sync.dma_start(
                    t[:], x_d.ap()[r0 + j * P:r0 + (j + 1) * P, :])
                xt.append(t)
            return xt

        def stage_front(sb, xt=None):
            """Transpose, split, mm1 -> returns (ph, pl)."""
            if xt is None:
                xt = load_x(sb)
            xh, xl = [], []
            for d in range(D_CH):
                pst = ps_tp.tile([P, ROWS_SB], f32, tag="ps_x", name=f"pst{sb}_{d}")
                for j in range(ROWS_SB // P):
                    nc.tensor.transpose(
                        pst[:, j * P:(j + 1) * P],
                        xt[j][:, d * P:(d + 1) * P], ident[:])
                h = xsplit.tile([P, ROWS_SB], f16, tag=f"xh{d}", name=f"xh{sb}_{d}")
                l = xsplit.tile([P, ROWS_SB], f16, tag=f"xl{d}", name=f"xl{sb}_{d}")
                nc.vector.tensor_copy(h[:], pst[:])
                nc.vector.tensor_tensor(l[:], pst[:], h[:], op=SUB)
                xh.append(h)
                xl.append(l)

            ph, pl = [], []
            for e in range(E_CH):
                ps1 = ps_p1.tile([P, ROWS_SB], f32, tag="ps1", name=f"ps1_{sb}_{e}")
                mms = (
                    [(rp_hi[d], xh[d]) for d in range(D_CH)]
                    + [(rp_hi[d], xl[d]) for d in range(D_CH)]
                    + [(rp_lo[d], xh[d]) for d in range(D_CH)]
                )
                for k, (w, m) in enumerate(mms):
                    nc.tensor.matmul(
                        ps1[:], w[:, e * P:(e + 1) * P], m[:],
                        start=(k == 0), stop=(k == len(mms) - 1))
                h = projp.tile([P, ROWS_SB], f16, tag=f"ph{e}", name=f"ph{sb}_{e}")
                l = projp.tile([P, ROWS_SB], f16, tag=f"pl{e}", name=f"pl{sb}_{e}")
                nc.vector.tensor_copy(h[:], ps1[:])
                nc.vector.tensor_tensor(l[:], ps1[:], h[:], op=SUB)
                ph.append(h)
                pl.append(l)
            return ph, pl

        def stage_back(sb, ph, pl, interleave=None):
            """mm2 + argmax + index DMA for super-block sb."""
            r0 = sb * ROWS_SB
            for rb in range(ROWS_SB // P):
                rows = slice(rb * P, (rb + 1) * P)
                simb = simp.tile([P, C], f32, tag="simb", name=f"simb{sb}_{rb}")
                for cc in range(C_CH):
                    if interleave and rb == 0 and cc in interleave:
                        interleave[cc]()
                    ps2 = ps_p2.tile([P, CC_W], f32, tag="ps2",
                                     name=f"ps2_{sb}_{rb}_{cc}")
                    mms = (
                        [(ph[e], cnT_hi[e]) for e in range(E_CH)]
                        + [(ph[e], cnT_lo[e]) for e in range(E_CH)]
                        + [(pl[e], cnT_hi[e]) for e in range(E_CH)]
                    )
                    for k, (pw, cm) in enumerate(mms):
                        nc.tensor.matmul(
                            ps2[:], pw[:, rows],
                            cm[:, cc * CC_W:(cc + 1) * CC_W],
                            start=(k == 0), stop=(k == len(mms) - 1))
                    nc.scalar.copy(simb[:, cc * CC_W:(cc + 1) * CC_W], ps2[:])

                mx = outp.tile([P, 8], f32, tag="mx", name=f"mx{sb}_{rb}")
                idx = outp.tile([P, 8], u32, tag="idx", name=f"idx{sb}_{rb}")
                nc.vector.max(mx[:], simb[:])
                nc.vector.max_index(idx[:], mx[:], simb[:])
                nc.sync.dma_start(
                    xq_d.ap()[r0 + rb * P:r0 + (rb + 1) * P, :], idx[:, 0:1])

        xt0 = load_x(0)
        rp_prologue()
        cb_prologue(range(0, 8))
        fronts = {0: stage_front(0, xt0)}
        cb_prologue(range(8, 16))
        fronts[1] = stage_front(1)
        cb_prologue(range(16, 24))
        for sb in range(N_SB):
            if sb + 2 in range(N_SB):
                fronts[sb + 2] = stage_front(sb + 2)
            if sb == 0:
                cb_prologue(range(24, 32))
            stage_back(sb, *fronts.pop(sb))

    nc.compile()
    return nc


def _get_program():
    global _PROG
    if _PROG is None:
        _PROG = _build_program()
    return _PROG


def kernel(x, random_projection, codebook, _trace=False):
    from concourse import bass_utils

    nc = _get_program()
    rp = np.ascontiguousarray(random_projection, dtype=np.float32)
    cb = np.ascontiguousarray(codebook, dtype=np.float32)
    in_maps = [
        {"x": np.ascontiguousarray(x[b], dtype=np.float32), "rp": rp, "cb": cb}
        for b in range(B)
    ]
    res = bass_utils.run_bass_kernel_spmd(
        nc, in_maps, core_ids=list(range(B)), trace=_trace)
    out = np.stack(
        [res.results[b]["xq"][:, 0].astype(np.int32) for b in range(B)])
    if _trace:
        kernel.last_results = res
    return out

